# revision 1
# baseline (speedup 1.0000x reference)
"""GCN (2-layer GraphConv + edge scorer) on 8 Trainium2 NeuronCores.

Strategy (dst-sharded graph parallel):
  - Nodes padded to 50176 = 8 cores x 49 blocks x 128; core i owns dst nodes
    [i*6272, (i+1)*6272).
  - Host sorts edges by (dst block, src half) and pads each (block, half)
    group to a whole number of 128-edge tiles (tile counts shared across
    cores = max over cores, so the single SPMD program fits all cores).
  - segment_sum commutes with the dense matmul:
        x1 = relu(((sum_e s_e * X[src_e]) @ W1) + b1),  s_e = rsqd_out[src]*rsqd_in[dst]
    so each core gathers raw feature rows for its edges (dma_gather, int16
    indices -> tables split in lo/hi halves at row 25088), builds a scaled
    one-hot indicator per 128-edge tile on the VectorEngine, and uses the
    TensorEngine to scatter-accumulate agg^T in PSUM. Dense matmul per
    128-node block follows; AllGather shares x1 across cores for layer 2.
  - Edge scores: s_src/s_dst per node are computed per block, replicated to
    32-wide fields in a [node, 64] table, AllGathered, and per-edge values
    fetched with two more dma_gathers; sigmoid on the ScalarEngine.
Host does index-only preprocessing (sorting, degree counts, padding) and
reassembles the per-core score tiles into the original edge order.
"""
import os
import sys

_REPO = os.environ.get("TRN_RL_REPO", "/opt/trn_rl_repo")
if _REPO not in sys.path:
    sys.path.insert(0, _REPO)

import numpy as np

import concourse.bass as bass
import concourse.bacc as bacc
import concourse.tile as tile
from concourse import mybir
from concourse.bass_utils import run_bass_kernel_spmd

P = 128
NCORES = 8
N_NODES = 50000
NPAD = 50176            # 8 * 49 * 128
BPC = NPAD // NCORES // P   # blocks per core = 49
HALF = NPAD // 2        # 25088, split point for int16 gather indices
IN_F = 256
HID = 256
OUT_F = 128

f32 = mybir.dt.float32
bf16 = mybir.dt.bfloat16
i16 = mybir.dt.int16
MAX_GT = 8   # dma_gather ucode limit: <=1024 indices per call


def _wrap_idx(idx_flat):
    """dma_gather index layout: idx k -> [k%16, k//16], replicated 8x to 128 partitions."""
    n = idx_flat.shape[0]
    w = idx_flat.reshape(n // 16, 16).T
    return np.tile(w, (8, 1)).astype(np.int16)


def build_program(T_lo, T_hi, trace_label="gcn"):
    """One SPMD program for all 8 cores. T_lo/T_hi: per-local-block tile counts."""
    NB = len(T_lo)
    NB_RUN = int(os.environ.get("GCN_NB_LIMIT", NB))
    PHASE = int(os.environ.get("GCN_PHASE", 5))
    BF16_X1 = os.environ.get("GCN_BF16", "0") == "1"
    x1dt = bf16 if BF16_X1 else f32
    TE = int(sum(T_lo) + sum(T_hi))          # total edge tiles per core
    NIc = 16 * TE * 8                        # idx cols = 8*T per call, laid per call

    nc = bacc.Bacc("TRN2", target_bir_lowering=False, debug=False,
                   enable_asserts=True, num_devices=NCORES)

    feat_lo = nc.dram_tensor("feat_lo", [HALF, IN_F], f32, kind="ExternalInput")
    feat_hi = nc.dram_tensor("feat_hi", [HALF, IN_F], f32, kind="ExternalInput")
    w1 = nc.dram_tensor("w1", [P, 2 * HID], f32, kind="ExternalInput")
    w2 = nc.dram_tensor("w2", [P, 2 * OUT_F], f32, kind="ExternalInput")
    b1r = nc.dram_tensor("b1r", [P, HID], f32, kind="ExternalInput")
    b2r = nc.dram_tensor("b2r", [P, OUT_F], f32, kind="ExternalInput")
    wpt = nc.dram_tensor("wpt", [P, OUT_F], f32, kind="ExternalInput")
    wpb = nc.dram_tensor("wpb", [P, OUT_F], f32, kind="ExternalInput")
    iota_d = nc.dram_tensor("iota", [P, P], f32, kind="ExternalInput")
    bp_d = nc.dram_tensor("bp", [P, 1], f32, kind="ExternalInput")
    src16_d = nc.dram_tensor("src16", [P, 8 * TE], i16, kind="ExternalInput")
    dst16_d = nc.dram_tensor("dst16", [P, 8 * TE], i16, kind="ExternalInput")
    col_d = nc.dram_tensor("colv", [P, TE], f32, kind="ExternalInput")
    scale_d = nc.dram_tensor("scalev", [P, TE], f32, kind="ExternalInput")
    scores_d = nc.dram_tensor("scores", [P, TE], f32, kind="ExternalOutput")

    with tile.TileContext(nc) as tc:
        with (
            tc.tile_pool(name="cons", bufs=1) as cons,
            tc.tile_pool(name="sb", bufs=2) as sb,
            tc.tile_pool(name="ps", bufs=2, space="PSUM") as ps,
            tc.tile_pool(name="dram", bufs=1, space="DRAM") as dr,
        ):
            # ---- resident constants / indices ----
            w1_sb = cons.tile([P, 2 * HID], f32, name="w1_sb")
            w2_sb = cons.tile([P, 2 * OUT_F], f32, name="w2_sb")
            b1_sb = cons.tile([P, HID], f32, name="b1_sb")
            b2_sb = cons.tile([P, OUT_F], f32, name="b2_sb")
            wpt_sb = cons.tile([P, OUT_F], f32, name="wpt_sb")
            wpb_sb = cons.tile([P, OUT_F], f32, name="wpb_sb")
            iota_sb = cons.tile([P, P], f32, name="iota_sb")
            bp_sb = cons.tile([P, 1], f32, name="bp_sb")
            src16 = cons.tile([P, 8 * TE], i16, name="src16")
            dst16 = cons.tile([P, 8 * TE], i16, name="dst16")
            colv = cons.tile([P, TE], f32, name="colv")
            scalev = cons.tile([P, TE], f32, name="scalev")
            for s_t, d_t in [(w1_sb, w1), (w2_sb, w2), (b1_sb, b1r), (b2_sb, b2r),
                             (wpt_sb, wpt), (wpb_sb, wpb), (iota_sb, iota_d),
                             (bp_sb, bp_d), (src16, src16_d), (dst16, dst16_d),
                             (colv, col_d), (scalev, scale_d)]:
                nc.sync.dma_start(s_t[:], d_t[:])

            # ---- DRAM intermediates ----
            x1_slice = dr.tile([BPC * P, HID], x1dt, name="x1_slice")
            x1_full = dr.tile([NPAD, HID], x1dt, name="x1_full")
            s_slice = dr.tile([BPC * P, 64], f32, name="s_slice")
            s_full = dr.tile([NPAD, 64], f32, name="s_full")

            def conv_layer(lo_tab, hi_tab, w_sb, b_sb, d_in, d_out, out_cb,
                           gdt=f32, out_dt=f32):
                """One GraphConv layer over all blocks. out_cb(b, x_sb) consumes
                the activated [P, d_out] block."""
                nch = d_in // P
                gt0 = 0
                for b in range(NB_RUN):
                    tl, th = int(T_lo[b]), int(T_hi[b])
                    T = tl + th
                    if T == 0:
                        aggT = sb.tile([P, nch * P], f32, tag="aggT", name="aggT")
                        nc.vector.memset(aggT[:], 0.0)
                    else:
                        gat = sb.tile([P, T, d_in], gdt, tag="gat",
                                      name="gat", bufs=3)
                        # idx cols per call: 128*T/16 = 8*T ; call offset = 8*gt0
                        for tab, slot0, nt in [(lo_tab, 0, tl), (hi_tab, tl, th)]:
                            done = 0
                            while done < nt:
                                n = min(MAX_GT, nt - done)
                                t0 = gt0 + slot0 + done
                                nc.gpsimd.dma_gather(
                                    gat[:, slot0 + done: slot0 + done + n, :], tab,
                                    src16[:, 8 * t0: 8 * (t0 + n)],
                                    P * n, P * n, d_in)
                                done += n
                        aggT_ps = [ps.tile([P, P], f32, tag=f"aggT_ps{c}",
                                           name=f"aggT_ps{c}") for c in range(nch)]
                        for t in range(T):
                            g = gt0 + t
                            ind = sb.tile([P, P], gdt, tag="ind", name="ind", bufs=3)
                            nc.vector.tensor_scalar(
                                out=ind[:], in0=iota_sb[:],
                                scalar1=colv[:, g:g + 1], scalar2=scalev[:, g:g + 1],
                                op0=mybir.AluOpType.is_equal, op1=mybir.AluOpType.mult)
                            for c in range(nch):
                                nc.tensor.matmul(
                                    out=aggT_ps[c][:],
                                    lhsT=gat[:, t, c * P:(c + 1) * P],
                                    rhs=ind[:],
                                    start=(t == 0), stop=(t == T - 1))
                        aggT = sb.tile([P, nch * P], f32, tag="aggT", name="aggT")
                        for c in range(nch):
                            nc.vector.tensor_copy(aggT[:, c * P:(c + 1) * P], aggT_ps[c][:])
                    x_ps = ps.tile([P, d_out], f32, tag="x_ps", name="x_ps")
                    for c in range(nch):
                        nc.tensor.matmul(
                            out=x_ps[:], lhsT=aggT[:, c * P:(c + 1) * P],
                            rhs=w_sb[:, c * d_out:(c + 1) * d_out],
                            start=(c == 0), stop=(c == nch - 1))
                    xb = sb.tile([P, d_out], f32, tag="xb", name="xb")
                    nc.vector.tensor_tensor(out=xb[:], in0=x_ps[:], in1=b_sb[:],
                                            op=mybir.AluOpType.add)
                    xr = sb.tile([P, d_out], out_dt, tag="xr", name="xr")
                    nc.scalar.activation(xr[:], xb[:], mybir.ActivationFunctionType.Relu)
                    out_cb(b, xr)
                    gt0 += T

            # ---- layer 1 ----
            def l1_out(b, xr):
                nc.sync.dma_start(x1_slice[b * P:(b + 1) * P, :], xr[:])
            conv_layer(feat_lo[:], feat_hi[:], w1_sb, b1_sb, IN_F, HID, l1_out,
                       gdt=f32, out_dt=x1dt)

            if PHASE >= 2:
                nc.gpsimd.collective_compute(
                    "AllGather", mybir.AluOpType.bypass,
                    replica_groups=[list(range(NCORES))],
                    ins=[x1_slice.opt()], outs=[x1_full.opt()])

            # ---- layer 2 + per-node scores ----
            def l2_out(b, xr):
                scr = sb.tile([P, OUT_F], f32, tag="scr", name="scr")
                s_src = sb.tile([P, 1], f32, tag="s_src", name="s_src")
                s_dst = sb.tile([P, 1], f32, tag="s_dst", name="s_dst")
                scr2 = sb.tile([P, OUT_F], f32, tag="scr2", name="scr2")
                nc.vector.tensor_tensor(out=scr[:], in0=xr[:], in1=wpt_sb[:],
                                        op=mybir.AluOpType.mult)
                nc.vector.tensor_reduce(out=s_src[:], in_=scr[:],
                                        op=mybir.AluOpType.add,
                                        axis=mybir.AxisListType.X)
                nc.vector.tensor_tensor(out=scr2[:], in0=xr[:], in1=wpb_sb[:],
                                        op=mybir.AluOpType.mult)
                nc.vector.tensor_reduce(out=s_dst[:], in_=scr2[:],
                                        op=mybir.AluOpType.add,
                                        axis=mybir.AxisListType.X)
                sblk = sb.tile([P, 64], f32, tag="sblk", name="sblk")
                nc.vector.tensor_copy(sblk[:, 0:32], s_src[:, 0:1].to_broadcast([P, 32]))
                nc.vector.tensor_copy(sblk[:, 32:64], s_dst[:, 0:1].to_broadcast([P, 32]))
                nc.sync.dma_start(s_slice[b * P:(b + 1) * P, :], sblk[:])
            if PHASE >= 3:
                conv_layer(x1_full[0:HALF, :], x1_full[HALF:NPAD, :],
                           w2_sb, b2_sb, HID, OUT_F, l2_out,
                           gdt=x1dt, out_dt=f32)

            if PHASE >= 4:
                nc.gpsimd.collective_compute(
                    "AllGather", mybir.AluOpType.bypass,
                    replica_groups=[list(range(NCORES))],
                    ins=[s_slice.opt()], outs=[s_full.opt()])

            # ---- edge scores ----
            gt0 = 0
            for b in (range(NB_RUN) if PHASE >= 5 else []):
                tl, th = int(T_lo[b]), int(T_hi[b])
                T = tl + th
                if T == 0:
                    continue
                gA = sb.tile([P, T, 64], f32, tag="gA", name="gA", bufs=3)
                gB = sb.tile([P, T, 64], f32, tag="gB", name="gB", bufs=3)
                for tab, slot0, nt in [(s_full[0:HALF, :], 0, tl),
                                       (s_full[HALF:NPAD, :], tl, th)]:
                    done = 0
                    while done < nt:
                        n = min(MAX_GT, nt - done)
                        t0 = gt0 + slot0 + done
                        nc.gpsimd.dma_gather(
                            gA[:, slot0 + done: slot0 + done + n, :], tab,
                            src16[:, 8 * t0: 8 * (t0 + n)], P * n, P * n, 64)
                        done += n
                done = 0
                while done < T:
                    n = min(MAX_GT, T - done)
                    t0 = gt0 + done
                    nc.gpsimd.dma_gather(
                        gB[:, done: done + n, :], s_slice[:],
                        dst16[:, 8 * t0: 8 * (t0 + n)], P * n, P * n, 64)
                    done += n
                lsum = sb.tile([P, T], f32, tag="lsum", name="lsum", bufs=3)
                nc.vector.tensor_tensor(
                    out=lsum[:],
                    in0=gA[:, :, 0],
                    in1=gB[:, :, 32],
                    op=mybir.AluOpType.add)
                sc = sb.tile([P, T], f32, tag="sc", name="sc", bufs=3)
                nc.scalar.activation(sc[:], lsum[:],
                                     mybir.ActivationFunctionType.Sigmoid,
                                     bias=bp_sb[:, 0:1])
                nc.sync.dma_start(scores_d[:, gt0:gt0 + T], sc[:])
                gt0 += T

    nc.compile()
    return nc


def preprocess(features, src, dst, W1, b1, W2, b2, Wp, bp):
    """Sort/pad edges, build per-core input maps + reassembly info."""
    E = src.shape[0]
    src = src.astype(np.int64)
    dst = dst.astype(np.int64)

    deg_out = np.bincount(src, minlength=N_NODES).astype(np.float64)
    deg_in = np.bincount(dst, minlength=N_NODES).astype(np.float64)
    rsq_out = (1.0 / np.sqrt(np.clip(deg_out, 1.0, None))).astype(np.float32)
    rsq_in = (1.0 / np.sqrt(np.clip(deg_in, 1.0, None))).astype(np.float32)
    scale_e = (rsq_out[src] * rsq_in[dst]).astype(np.float32)

    gblk = dst // P                     # global block, 0..391
    half = (src >= HALF).astype(np.int64)
    key = gblk * 2 + half
    order = np.argsort(key, kind="stable")
    key_s = key[order]
    # group boundaries for all 392*2 groups
    bounds = np.searchsorted(key_s, np.arange(2 * (NPAD // P) + 1))

    cnt = np.diff(bounds)               # per (gblk, half)
    cnt2 = cnt.reshape(NPAD // P, 2)    # [392, 2]
    # per-core local blocks: global g = core*49 + b
    cnt3 = cnt2.reshape(NCORES, BPC, 2)
    T_lo = np.ceil(cnt3[:, :, 0].max(axis=0) / P).astype(np.int64)
    T_hi = np.ceil(cnt3[:, :, 1].max(axis=0) / P).astype(np.int64)
    TE = int(T_lo.sum() + T_hi.sum())

    src_s = src[order]
    dst_s = dst[order]
    scale_s = scale_e[order]

    # per-core slot arrays
    slot_src = np.zeros((NCORES, TE * P), np.int64)
    slot_dst = np.zeros((NCORES, TE * P), np.int64)   # pad dst -> core base
    slot_col = np.zeros((NCORES, TE * P), np.float32)
    slot_scale = np.zeros((NCORES, TE * P), np.float32)
    slot_orig = np.full((NCORES, TE * P), -1, np.int64)

    # slot offset of each (b, half) call
    call_off = np.zeros((BPC, 2), np.int64)
    off = 0
    for b in range(BPC):
        call_off[b, 0] = off
        off += int(T_lo[b]) * P
        call_off[b, 1] = off
        off += int(T_hi[b]) * P
    assert off == TE * P

    for core in range(NCORES):
        base = core * BPC * P
        for b in range(BPC):
            g = core * BPC + b
            for h in (0, 1):
                lo_e, hi_e = bounds[2 * g + h], bounds[2 * g + h + 1]
                n = hi_e - lo_e
                o = call_off[b, h]
                slot_src[core, o:o + n] = src_s[lo_e:hi_e]
                slot_dst[core, o:o + n] = dst_s[lo_e:hi_e]
                slot_col[core, o:o + n] = (dst_s[lo_e:hi_e] - g * P).astype(np.float32)
                slot_scale[core, o:o + n] = scale_s[lo_e:hi_e]
                slot_orig[core, o:o + n] = order[lo_e:hi_e]
                # pads: src=0 (idx 0 in its half table), dst=core base, scale=0
                pad_n = (int(T_lo[b]) if h == 0 else int(T_hi[b])) * P - n
                if pad_n:
                    slot_dst[core, o + n:o + n + pad_n] = base

    # per-core device arrays
    in_maps = []
    feat_pad = np.zeros((NPAD, IN_F), np.float32)
    feat_pad[:N_NODES] = features
    feat_lo = np.ascontiguousarray(feat_pad[:HALF])
    feat_hi = np.ascontiguousarray(feat_pad[HALF:])
    w1c = np.concatenate([W1[:P, :], W1[P:, :]], axis=1).astype(np.float32)
    w2c = np.concatenate([W2[:P, :], W2[P:, :]], axis=1).astype(np.float32)
    b1_rep = np.broadcast_to(b1.astype(np.float32)[None, :], (P, HID)).copy()
    b2_rep = np.broadcast_to(b2.astype(np.float32)[None, :], (P, OUT_F)).copy()
    wpt_rep = np.broadcast_to(Wp[:OUT_F, 0].astype(np.float32)[None, :], (P, OUT_F)).copy()
    wpb_rep = np.broadcast_to(Wp[OUT_F:, 0].astype(np.float32)[None, :], (P, OUT_F)).copy()
    iota = np.broadcast_to(np.arange(P, dtype=np.float32)[None, :], (P, P)).copy()
    bp_t = np.full((P, 1), np.float32(bp[0]))

    for core in range(NCORES):
        ssrc = slot_src[core]
        s16 = np.where(ssrc >= HALF, ssrc - HALF, ssrc)
        d16 = slot_dst[core] - core * BPC * P
        in_maps.append(dict(
            feat_lo=feat_lo, feat_hi=feat_hi, w1=w1c, w2=w2c,
            b1r=b1_rep, b2r=b2_rep, wpt=wpt_rep, wpb=wpb_rep,
            iota=iota, bp=bp_t,
            src16=_wrap_idx(s16), dst16=_wrap_idx(d16),
            colv=np.ascontiguousarray(slot_col[core].reshape(TE, P).T),
            scalev=np.ascontiguousarray(slot_scale[core].reshape(TE, P).T),
        ))

    return in_maps, slot_orig, T_lo, T_hi, E


_CACHE = {}


def _get_program(T_lo, T_hi):
    key = (tuple(T_lo), tuple(T_hi), os.environ.get("GCN_BF16", "0"))
    if key not in _CACHE:
        _CACHE[key] = build_program(T_lo, T_hi)
    return _CACHE[key]


def kernel(features, src, dst, edge_type, W1, b1, W2, b2, Wp, bp, _trace=False,
           _tmpdir=None):
    features = np.asarray(features, np.float32)
    src_i = np.asarray(src, np.int32)
    dst_i = np.asarray(dst, np.int32)
    in_maps, slot_orig, T_lo, T_hi, E = preprocess(
        features, src_i, dst_i, np.asarray(W1), np.asarray(b1),
        np.asarray(W2), np.asarray(b2), np.asarray(Wp), np.asarray(bp))
    nc = _get_program(T_lo, T_hi)
    res = run_bass_kernel_spmd(nc, in_maps, core_ids=list(range(NCORES)),
                               trace=_trace, tmpdir=_tmpdir)
    out = np.zeros(E, np.float32)
    for core in range(NCORES):
        sc = res.results[core]["scores"]        # [P, TE]
        flat = sc.T.reshape(-1)                 # slot q = tile*128+p -> [tile, p]
        so = slot_orig[core]
        m = so >= 0
        out[so[m]] = flat[m]
    if _trace:
        kernel._last_results = res
    return out



# revision 8
# speedup vs baseline: 1.0794x; 1.0794x over previous
"""GCN (2-layer GraphConv + edge scorer) on 8 Trainium2 NeuronCores — v2.

Strategy (dst-sharded, descriptor-gen minimized):
  - Nodes padded to 50176 = 8 x 49 x 128; core i owns dst nodes
    [i*6272, (i+1)*6272). Edges sorted (dst block, src half); per-(block,
    half) groups padded to whole 128-edge tiles (counts = max over cores so
    one SPMD program fits all).
  - Normalization split: rsq_out folded into X on the host, rsq_in applied
    post-aggregation per dst block. One-hot scatter tiles become pure 0/1,
    host-precomputed in fp8 and streamed (no per-tile DVE indicator build).
  - h1 = (X*rsq_out)@W1 computed per owned block, AllGathered in bf16;
    per-edge rows fetched with dma_gather in prepare_only mode +
    trigger_dma so the Pool engine only pays descriptor-gen, not the DMA
    drain. TensorE scatter-accumulates aggT per block in PSUM.
  - Layer 2 gathers h2 = (x1*rsq_out)@W2 rows (128-wide bf16), so gather
    traffic and scatter matmuls are halved vs gathering x1.
  - Scores: s_src/s_dst per node from x2T via one matmul; s_src replicated
    into a [node,128]-bf16 table, AllGathered, fetched per edge with a
    transpose-mode gather (edges land on the free dim); s_dst expanded
    per edge locally via matmul with host-streamed transposed one-hot
    tiles. sigmoid(+bp) on ScalarE; row 0 of the result is the output.
Host does index-only preprocessing and reassembles per-core score slots
into the original edge order.
"""
import os
import sys

_REPO = os.environ.get("TRN_RL_REPO", "/opt/trn_rl_repo")
if _REPO not in sys.path:
    sys.path.insert(0, _REPO)

import ml_dtypes
import numpy as np

import concourse.bacc as bacc
import concourse.tile as tile
from concourse import mybir
from concourse.bass_utils import run_bass_kernel_spmd

P = 128
NCORES = 8
N_NODES = 50000
NPAD = 50176            # 8 * 49 * 128
BPC = NPAD // NCORES // P   # blocks per core = 49
HALF = NPAD // 2        # int16 gather index limit -> lo/hi table split
IN_F = 256
HID = 256
OUT_F = 128
MAX_GT = int(os.environ.get("GCN2_MAXGT", "8"))   # tiles per gather call

f32 = mybir.dt.float32
bf16 = mybir.dt.bfloat16
i16 = mybir.dt.int16
fp8 = mybir.dt.float8e4
IND_BF16 = os.environ.get("GCN2_IND_BF16", "0") == "1"
ind_dt = bf16 if IND_BF16 else fp8
D_FALLBACK = os.environ.get("GCN2_D_FALLBACK", "1") == "1"
NSEM = 8


def _wrap_idx(idx_flat):
    """dma_gather index layout: idx k -> [k%16, k//16], replicated 8x."""
    n = idx_flat.shape[0]
    w = idx_flat.reshape(n // 16, 16).T
    return np.tile(w, (8, 1)).astype(np.int16)


def build_program(T_arr, trace_label="gcn2"):
    """One SPMD program for all cores. T_arr: [BPC, 2] per-(block, half)
    tile counts (shared across cores)."""
    NB = T_arr.shape[0]
    TE = int(T_arr.sum())                 # total edge tiles per core
    NL = NB * P                           # nodes per core
    PHASE = int(os.environ.get("GCN2_PHASE", "4"))

    nc = bacc.Bacc("TRN2", target_bir_lowering=False, debug=False,
                   enable_asserts=True, num_devices=NCORES)

    xT_d = nc.dram_tensor("xT", [P, NB * 2 * P], bf16, kind="ExternalInput")
    w1_d = nc.dram_tensor("w1", [P, 2 * HID], bf16, kind="ExternalInput")
    w2_d = nc.dram_tensor("w2", [P, 2 * OUT_F], bf16, kind="ExternalInput")
    wp_d = nc.dram_tensor("wp12", [P, 2], f32, kind="ExternalInput")
    b1_d = nc.dram_tensor("b1c", [P, 2], f32, kind="ExternalInput")
    b2_d = nc.dram_tensor("b2c", [P, 1], f32, kind="ExternalInput")
    bp_d = nc.dram_tensor("bp", [P, 1], f32, kind="ExternalInput")
    rsqi_d = nc.dram_tensor("rsqi", [P, NL], f32, kind="ExternalInput")
    rsqo_d = nc.dram_tensor("rsqo", [P, NB], f32, kind="ExternalInput")
    src16_d = nc.dram_tensor("src16", [P, 8 * TE], i16, kind="ExternalInput")
    ind_d = nc.dram_tensor("ind8", [P, TE * P], ind_dt, kind="ExternalInput")
    indT_d = nc.dram_tensor("indT8", [P, TE * P], ind_dt, kind="ExternalInput")
    dst16_d = nc.dram_tensor("dst16", [P, 8 * TE], i16, kind="ExternalInput")
    scores_d = nc.dram_tensor("scores", [1, TE * P], f32, kind="ExternalOutput")
    scoresPT_d = nc.dram_tensor("scoresPT", [P, TE], f32, kind="ExternalOutput")

    with tile.TileContext(nc) as tc:
        with (
            tc.tile_pool(name="cons", bufs=1) as cons,
            tc.tile_pool(name="sb", bufs=2) as sb,
            tc.tile_pool(name="gp", bufs=3) as gp,
            tc.tile_pool(name="ps", bufs=2, space="PSUM") as ps,
            tc.tile_pool(name="psmm", bufs=2, space="PSUM") as psmm,
            tc.tile_pool(name="dram", bufs=1, space="DRAM") as dr,
        ):
            # ---- resident constants ----
            w1_sb = cons.tile([P, 2 * HID], bf16, name="w1_sb")
            w2_sb = cons.tile([P, 2 * OUT_F], bf16, name="w2_sb")
            wp_sb = cons.tile([P, 2], f32, name="wp_sb")
            b1_sb = cons.tile([P, 2], f32, name="b1_sb")
            b2_sb = cons.tile([P, 1], f32, name="b2_sb")
            bp_sb = cons.tile([P, 1], f32, name="bp_sb")
            rsqi_sb = cons.tile([P, NL], f32, name="rsqi_sb")
            rsqo_sb = cons.tile([P, NB], f32, name="rsqo_sb")
            src16 = cons.tile([P, 8 * TE], i16, name="src16")
            dst16 = cons.tile([P, 8 * TE], i16, name="dst16")
            if D_FALLBACK:
                nc.sync.dma_start(dst16[:], dst16_d[:])
            sdr = cons.tile([P, NB * P], bf16, name="sdr")
            for s_t, d_t in [(w1_sb, w1_d), (w2_sb, w2_d), (wp_sb, wp_d),
                             (b1_sb, b1_d), (b2_sb, b2_d), (bp_sb, bp_d),
                             (rsqi_sb, rsqi_d), (rsqo_sb, rsqo_d),
                             (src16, src16_d)]:
                nc.sync.dma_start(s_t[:], d_t[:])

            # ---- DRAM intermediates ----
            h1_sl = dr.tile([NL, HID], bf16, name="h1_sl")
            h1_f = dr.tile([NPAD, HID], bf16, name="h1_f")
            h2_sl = dr.tile([NL, OUT_F], bf16, name="h2_sl")
            h2_f = dr.tile([NPAD, OUT_F], bf16, name="h2_f")
            s_sl = dr.tile([NL, P], bf16, name="s_sl")
            s_f = dr.tile([NPAD, P], bf16, name="s_f")

            # ---- gather helper (plain SWDGE gather, Tile-managed sync) ----
            def do_gather(dst_ap, tab_ap, idx_cols0, n_idx, **kw):
                nc.gpsimd.dma_gather(
                    dst_ap, tab_ap, src16[:, idx_cols0:idx_cols0 + n_idx // 16],
                    n_idx, n_idx, tab_ap.ap[-1][1], **kw)

            # ---- Phase A: h1 rows for owned nodes ----
            for b in range(NB):
                xt = sb.tile([P, 2 * P], bf16, tag="xt", name="xt")
                nc.sync.dma_start(xt[:], xT_d[:, b * 2 * P:(b + 1) * 2 * P])
                h1mm = psmm.tile([P, 4 * P], f32, tag="mm", name="h1mm")
                h1ps = h1mm[:, 0:HID]
                for c in range(2):
                    nc.tensor.matmul(
                        out=h1ps, lhsT=xt[:, c * P:(c + 1) * P],
                        rhs=w1_sb[:, c * HID:(c + 1) * HID],
                        start=(c == 0), stop=(c == 1))
                h1sb = sb.tile([P, HID], bf16, tag="h1sb", name="h1sb")
                nc.vector.tensor_copy(h1sb[:], h1ps)
                nc.sync.dma_start(h1_sl[b * P:(b + 1) * P, :], h1sb[:])

            nc.gpsimd.collective_compute(
                "AllGather", mybir.AluOpType.bypass,
                replica_groups=[list(range(NCORES))],
                ins=[h1_sl.opt()], outs=[h1_f.opt()])

            def conv_phase(tab_full, d_in, per_block_cb):
                """Gather+scatter over all (block, half) groups.
                per_block_cb(b, aggT_ps_chunks) consumes the accumulated
                block aggregation (list of [P, P] psum chunks)."""
                nch = d_in // P
                gt0 = 0
                for b in range(NB):
                    T_blk = int(T_arr[b, 0] + T_arr[b, 1])
                    aggT_ps = [ps.tile([P, P], f32, tag=f"aggT{c}",
                                       name=f"aggT{c}") for c in range(nch)]
                    tdone = 0
                    for h in range(2):
                        T = int(T_arr[b, h])
                        tab = tab_full[h * HALF:(h + 1) * HALF, :]
                        done = 0
                        while done < T:
                            n = min(MAX_GT, T - done)
                            t0 = gt0 + tdone + done
                            g = gp.tile([P, MAX_GT, d_in], bf16, tag="g",
                                        name="g")
                            do_gather(g[:, 0:n, :], tab, 8 * t0, P * n)
                            iw = gp.tile([P, MAX_GT * P], ind_dt, tag="iw",
                                         name="iw")
                            nc.sync.dma_start(
                                iw[:, 0:n * P],
                                ind_d[:, t0 * P:(t0 + n) * P])
                            for tt in range(n):
                                gl = tdone + done + tt
                                for c in range(nch):
                                    nc.tensor.matmul(
                                        out=aggT_ps[c][:],
                                        lhsT=g[:, tt, c * P:(c + 1) * P],
                                        rhs=iw[:, tt * P:(tt + 1) * P],
                                        start=(gl == 0),
                                        stop=(gl == T_blk - 1))
                            done += n
                        tdone += T
                    if T_blk == 0:
                        for c in range(nch):
                            nc.vector.memset(aggT_ps[c][:], 0.0)
                    per_block_cb(b, aggT_ps)
                    gt0 += T_blk

            # ---- Phase B: layer 1 + h2 rows ----
            def l1_block(b, aggT_ps):
                x1T = []
                for c in range(2):
                    z = sb.tile([P, P], f32, tag=f"z{c}", name=f"z{c}")
                    nc.vector.tensor_tensor(
                        out=z[:], in0=aggT_ps[c][:],
                        in1=rsqi_sb[:, b * P:(b + 1) * P],
                        op=mybir.AluOpType.mult)
                    xc = sb.tile([P, P], bf16, tag=f"x1T{c}", name=f"x1T{c}")
                    nc.scalar.activation(xc[:], z[:],
                                         mybir.ActivationFunctionType.Relu,
                                         bias=b1_sb[:, c:c + 1])
                    x1T.append(xc)
                h2mm = psmm.tile([P, 4 * P], f32, tag="mm", name="h2mm")
                h2ps = h2mm[:, 0:OUT_F]
                for c in range(2):
                    nc.tensor.matmul(
                        out=h2ps, lhsT=x1T[c][:],
                        rhs=w2_sb[:, c * OUT_F:(c + 1) * OUT_F],
                        start=(c == 0), stop=(c == 1))
                h2sb = sb.tile([P, OUT_F], bf16, tag="h2sb", name="h2sb")
                nc.vector.tensor_scalar(
                    out=h2sb[:], in0=h2ps, scalar1=rsqo_sb[:, b:b + 1],
                    scalar2=None, op0=mybir.AluOpType.mult)
                nc.sync.dma_start(h2_sl[b * P:(b + 1) * P, :], h2sb[:])

            if PHASE >= 2:
                conv_phase(h1_f[:], HID, l1_block)

            if PHASE >= 2:
                nc.gpsimd.collective_compute(
                    "AllGather", mybir.AluOpType.bypass,
                    replica_groups=[list(range(NCORES))],
                    ins=[h2_sl.opt()], outs=[h2_f.opt()])

            # ---- Phase C: layer 2 + per-node scores ----
            def l2_block(b, aggT_ps):
                z = sb.tile([P, P], f32, tag="z2", name="z2")
                nc.vector.tensor_tensor(
                    out=z[:], in0=aggT_ps[0][:],
                    in1=rsqi_sb[:, b * P:(b + 1) * P],
                    op=mybir.AluOpType.mult)
                x2T = sb.tile([P, P], f32, tag="x2T", name="x2T")
                nc.scalar.activation(x2T[:], z[:],
                                     mybir.ActivationFunctionType.Relu,
                                     bias=b2_sb[:, 0:1])
                smm = psmm.tile([P, 4 * P], f32, tag="mm", name="smm")
                sps = smm[:, 0:2]
                nc.tensor.matmul(out=sps, lhsT=x2T[:], rhs=wp_sb[:],
                                 start=True, stop=True)
                scp = sb.tile([P, 2], f32, tag="scp", name="scp")
                nc.vector.tensor_copy(scp[:], sps)
                srow = sb.tile([P, P], bf16, tag="srow", name="srow")
                if D_FALLBACK:
                    nc.vector.tensor_copy(srow[:, 0:64],
                                          scp[:, 0:1].to_broadcast([P, 64]))
                    nc.vector.tensor_copy(srow[:, 64:P],
                                          scp[:, 1:2].to_broadcast([P, 64]))
                else:
                    nc.vector.tensor_copy(srow[:],
                                          scp[:, 0:1].to_broadcast([P, P]))
                nc.sync.dma_start(s_sl[b * P:(b + 1) * P, :], srow[:])
                nc.vector.tensor_copy(sdr[:, b * P:(b + 1) * P],
                                      scp[:, 1:2].to_broadcast([P, P]))

            if PHASE >= 3:
                conv_phase(h2_f[:], OUT_F, l2_block)
                nc.gpsimd.collective_compute(
                    "AllGather", mybir.AluOpType.bypass,
                    replica_groups=[list(range(NCORES))],
                    ins=[s_sl.opt()], outs=[s_f.opt()])

            # ---- Phase D (fallback): baseline-style double gather ----
            gt0 = 0
            for b in (range(NB) if (PHASE >= 4 and D_FALLBACK) else []):
                T_blk = int(T_arr[b, 0] + T_arr[b, 1])
                if T_blk == 0:
                    continue
                gA = gp.tile([P, T_blk, P], bf16, tag="gA", name="gA")
                gB = gp.tile([P, T_blk, P], bf16, tag="gB", name="gB")
                tdone = 0
                for h in range(2):
                    T = int(T_arr[b, h])
                    tab = s_f[h * HALF:(h + 1) * HALF, :]
                    done = 0
                    while done < T:
                        n = min(MAX_GT, T - done)
                        t0 = gt0 + tdone + done
                        sl = tdone + done
                        nc.gpsimd.dma_gather(
                            gA[:, sl:sl + n, :], tab,
                            src16[:, 8 * t0:8 * (t0 + n)], P * n, P * n, P)
                        done += n
                    tdone += T
                done = 0
                while done < T_blk:
                    n = min(MAX_GT, T_blk - done)
                    t0 = gt0 + done
                    nc.gpsimd.dma_gather(
                        gB[:, done:done + n, :], s_sl[:],
                        dst16[:, 8 * t0:8 * (t0 + n)], P * n, P * n, P)
                    done += n
                lsum = sb.tile([P, MAX_GT * 4], f32, tag="lsum", name="lsum")
                nc.vector.tensor_tensor(
                    out=lsum[:, 0:T_blk], in0=gA[:, :, 0], in1=gB[:, :, 64],
                    op=mybir.AluOpType.add)
                scf = sb.tile([P, MAX_GT * 4], f32, tag="scf", name="scf")
                nc.scalar.activation(scf[:, 0:T_blk], lsum[:, 0:T_blk],
                                     mybir.ActivationFunctionType.Sigmoid,
                                     bias=bp_sb[:, 0:1])
                # output layout: [P, T] per block at tile offset gt0
                nc.sync.dma_start(
                    scoresPT_d[:, gt0:gt0 + T_blk], scf[:, 0:T_blk])
                gt0 += T_blk

            # ---- Phase D: per-edge scores (transpose-gather path) ----
            gt0 = 0
            for b in (range(NB) if (PHASE >= 4 and not D_FALLBACK) else []):
                for h in range(2):
                    T = int(T_arr[b, h])
                    tab = s_f[h * HALF:(h + 1) * HALF, :]
                    done = 0
                    while done < T:
                        n = min(MAX_GT, T - done)
                        t0 = gt0 + done
                        gT = gp.tile([P, 1, MAX_GT * P], bf16, tag="gT",
                                     name="gT")
                        do_gather(gT[:, :, 0:n * P], tab, 8 * t0, P * n,
                                  transpose=True)
                        iw = gp.tile([P, MAX_GT * P], ind_dt, tag="iwT",
                                     name="iwT")
                        nc.sync.dma_start(iw[:, 0:n * P],
                                          indT_d[:, t0 * P:(t0 + n) * P])
                        off = 0
                        while off < n * P:
                            w = min(4 * P, n * P - off)
                            sps2 = psmm.tile([P, 4 * P], f32, tag="mm",
                                             name="sps2")
                            nc.tensor.matmul(
                                out=sps2[:, 0:w],
                                lhsT=sdr[:, b * P:(b + 1) * P],
                                rhs=iw[:, off:off + w],
                                start=True, stop=True)
                            sc1 = sb.tile([P, 4 * P], f32, tag="sc1",
                                          name="sc1")
                            nc.vector.tensor_tensor(
                                out=sc1[:, 0:w], in0=sps2[:, 0:w],
                                in1=gT[:, 0, off:off + w],
                                op=mybir.AluOpType.add)
                            sc2 = sb.tile([P, 4 * P], f32, tag="sc2",
                                          name="sc2")
                            nc.scalar.activation(
                                sc2[:, 0:w], sc1[:, 0:w],
                                mybir.ActivationFunctionType.Sigmoid,
                                bias=bp_sb[:, 0:1])
                            nc.sync.dma_start(
                                scores_d[0:1,
                                         t0 * P + off:t0 * P + off + w],
                                sc2[0:1, 0:w])
                            off += w
                        done += n
                    gt0 += T

    nc.compile()
    return nc


def preprocess(features, src, dst, W1, b1, W2, b2, Wp, bp):
    """Sort/pad edges, build per-core input maps + reassembly info."""
    E = src.shape[0]
    src = src.astype(np.int64)
    dst = dst.astype(np.int64)
    n_nodes = features.shape[0]

    deg_out = np.bincount(src, minlength=n_nodes).astype(np.float64)
    deg_in = np.bincount(dst, minlength=n_nodes).astype(np.float64)
    rsq_out = (1.0 / np.sqrt(np.clip(deg_out, 1.0, None))).astype(np.float32)
    rsq_in = (1.0 / np.sqrt(np.clip(deg_in, 1.0, None))).astype(np.float32)

    NB = BPC
    NL = NB * P

    # edge sort: (dst block, src half, ...)
    gblk = dst // P
    half = (src >= HALF).astype(np.int64)
    key = gblk * 2 + half
    order = np.argsort(key, kind="stable")
    key_s = key[order]
    bounds = np.searchsorted(key_s, np.arange(2 * (NPAD // P) + 1))
    cnt = np.diff(bounds).reshape(NCORES, NB, 2)
    T_arr = np.ceil(cnt.max(axis=0) / P).astype(np.int64)   # [NB, 2]
    TE = int(T_arr.sum())

    src_s = src[order]
    dst_s = dst[order]

    # slot offset of each (b, h) group
    goff = np.zeros((NB, 2), np.int64)
    off = 0
    for b in range(NB):
        for h in range(2):
            goff[b, h] = off
            off += int(T_arr[b, h]) * P
    assert off == TE * P

    slot_src = np.zeros((NCORES, TE * P), np.int64)
    slot_col = np.full((NCORES, TE * P), -1, np.int64)   # -1 = pad
    slot_orig = np.full((NCORES, TE * P), -1, np.int64)

    for core in range(NCORES):
        for b in range(NB):
            g = core * NB + b
            for h in (0, 1):
                lo_e, hi_e = bounds[2 * g + h], bounds[2 * g + h + 1]
                nn = hi_e - lo_e
                o = goff[b, h]
                slot_src[core, o:o + nn] = src_s[lo_e:hi_e]
                slot_col[core, o:o + nn] = dst_s[lo_e:hi_e] - (core * NL + b * P)
                slot_orig[core, o:o + nn] = order[lo_e:hi_e]

    # host-side tensors
    xpad = np.zeros((NPAD, IN_F), np.float32)
    xpad[:n_nodes] = features * rsq_out[:, None]
    rsqi_pad = np.zeros(NPAD, np.float32)
    rsqi_pad[:n_nodes] = rsq_in
    rsqo_pad = np.zeros(NPAD, np.float32)
    rsqo_pad[:n_nodes] = rsq_out

    w1h = np.concatenate([W1[:P, :], W1[P:, :]], axis=1).astype(ml_dtypes.bfloat16)
    w2h = np.concatenate([W2[:P, :], W2[P:, :]], axis=1).astype(ml_dtypes.bfloat16)
    wp12 = np.stack([Wp[:OUT_F, 0], Wp[OUT_F:, 0]], axis=1).astype(np.float32)
    b1c = np.stack([b1[:P], b1[P:]], axis=1).astype(np.float32)
    b2c = b2.astype(np.float32)[:, None]
    bpc = np.full((P, 1), np.float32(bp[0]))

    lane = np.arange(TE * P) % P
    tidx = np.arange(TE * P) // P

    in_maps = []
    for core in range(NCORES):
        base = core * NL
        xT = np.zeros((P, NB * 2 * P), ml_dtypes.bfloat16)
        for b in range(NB):
            blk = xpad[base + b * P: base + (b + 1) * P]   # [128, 256]
            for c in range(2):
                xT[:, (b * 2 + c) * P:(b * 2 + c + 1) * P] = \
                    blk[:, c * P:(c + 1) * P].T.astype(ml_dtypes.bfloat16)
        rsqi_rep = np.broadcast_to(rsqi_pad[base:base + NL][None, :],
                                   (P, NL)).astype(np.float32).copy()
        rsqo_col = rsqo_pad[base:base + NL].reshape(NB, P).T.copy()

        scol = slot_col[core]
        valid = scol >= 0
        hdt = ml_dtypes.bfloat16 if IND_BF16 else ml_dtypes.float8_e4m3
        ind8 = np.zeros((P, TE * P), hdt)
        indT8 = np.zeros((P, TE * P), hdt)
        ind8[lane[valid], tidx[valid] * P + scol[valid]] = 1.0
        indT8[scol[valid], tidx[valid] * P + lane[valid]] = 1.0

        ssrc = slot_src[core]
        s16 = np.where(ssrc >= HALF, ssrc - HALF, ssrc)
        # local dst node index per slot (block * 128 + col); pads -> 0
        blk_of_slot = np.zeros(TE * P, np.int64)
        off2 = 0
        for b in range(NB):
            w2_ = int(T_arr[b].sum()) * P
            blk_of_slot[off2:off2 + w2_] = b
            off2 += w2_
        dloc = np.where(valid, blk_of_slot * P + np.maximum(scol, 0), 0)
        in_maps.append(dict(
            xT=xT, w1=w1h, w2=w2h, wp12=wp12, b1c=b1c, b2c=b2c, bp=bpc,
            rsqi=rsqi_rep, rsqo=rsqo_col,
            src16=_wrap_idx(s16), dst16=_wrap_idx(dloc),
            ind8=ind8, indT8=indT8,
        ))

    return in_maps, slot_orig, T_arr, E


_CACHE = {}


def _get_program(T_arr):
    key = (tuple(map(tuple, T_arr)), os.environ.get("GCN2_PHASE", "4"), MAX_GT,
           IND_BF16, D_FALLBACK)
    if key not in _CACHE:
        _CACHE[key] = build_program(np.asarray(T_arr))
    return _CACHE[key]


def kernel(features, src, dst, edge_type, W1, b1, W2, b2, Wp, bp, _trace=False,
           _tmpdir=None):
    features = np.asarray(features, np.float32)
    src_i = np.asarray(src, np.int32)
    dst_i = np.asarray(dst, np.int32)
    in_maps, slot_orig, T_arr, E = preprocess(
        features, src_i, dst_i, np.asarray(W1), np.asarray(b1),
        np.asarray(W2), np.asarray(b2), np.asarray(Wp), np.asarray(bp))
    nc = _get_program(T_arr)
    res = run_bass_kernel_spmd(nc, in_maps, core_ids=list(range(NCORES)),
                               trace=_trace, tmpdir=_tmpdir)
    out = np.zeros(E, np.float32)
    for core in range(NCORES):
        if D_FALLBACK:
            sc = np.asarray(res.results[core]["scoresPT"])   # [P, TE]
            flat = sc.T.reshape(-1)
        else:
            sc = res.results[core]["scores"]    # [1, TE*P]
            flat = np.asarray(sc).reshape(-1)
        so = slot_orig[core]
        m = so >= 0
        out[so[m]] = flat[m[:flat.shape[0]] if m.shape[0] > flat.shape[0] else m]
    if _trace:
        kernel._last_results = res
    return out
